# revision 82
# baseline (speedup 1.0000x reference)
"""Bass/Trainium2 kernel for nn_Attention_21354577395789 (fp8 DoubleRow).

Reference computation (B=16, S=2048, H=1024, D=2H=2048):
    h      = broadcast(hidden[1, 2H]) -> [B, S, 2H]
    cat    = concat([h, enc], -1)                    [B, S, 4H]
    energy = tanh(cat @ attn_w.T + attn_b)           [B, S, H]
    scores = energy @ v_w.T                          [B, S, 1]
    attn   = softmax(scores, axis=1)
    ctx    = attn^T @ enc                            [B, 1, 2H]

Algebraic simplifications (as the bf16 baseline):
  * attn_w = [W_h | W_e]; c = hidden @ W_h.T + attn_b is a single [H]
    vector computed on the host.
  * streaming softmax without max subtraction; per-chunk unnormalized
    context partials + exp-sums; final reduction on host.

Precision strategy (the speedup over the bf16 baseline):
  * The O(S*D*H) energy matmul runs in fp8-e4m3 with
    perf_mode=DoubleRow: two k-slices (256 contraction rows) per PE
    pass -> ~1.8x the bf16 matmul rate. W_e is pre-scaled by WS=64 on
    the host (its std 1/64 would land in e4m3's subnormal range);
    the tanh activation applies scale=1/WS to undo it.
  * Optionally the last NKK16 k-tiles run as plain fp16 matmuls into
    the same PSUM accumulation, dialing the quantization error down
    (sim: NKK16=0 -> 1.78e-2, 2 -> 1.68e-2, 4 -> 1.55e-2 vs the 2e-2
    gate) at ~267ns per extra matmul.
  * The context path (DVE) reads a separate fp16 copy of enc, so fp8
    noise never touches the context accumulation.

Engine placement per 512-chunk of S:
  PE    : DoubleRow energy matmuls (the only O(S*D*H) work) + one
          ones-stationary score-reduce per chunk (carry-deferred into
          the next chunk's jj=1 slot)
  ACT   : tanh(+bias+1/WS scale), exp(+chunk sum)
  GpSimd: broadcast of the exp row across partitions
  DVE   : v-mult + jj-tree (fp16) and 16 per-kk context stt ops,
          issued 3 per jj slot so the strict-FIFO DVE queue never
          starves the ev/add chain that recycles tanh buffers
The last batch's final three chunks skip the DVE context path
entirely: their exp rows are DMA-transposed into partition columns
and the context partials run as PE matmuls against a [t, d]-layout
fp16 enc copy, accumulating in one PSUM readout. This keeps the
end-of-stream region (where per-chunk PE time shrinks) free of the
DVE backlog that otherwise convoys the in-order PE queue.

Distribution: data-parallel over B across 8 NeuronCores (2 batches
per core), no collectives.
"""

import os

import numpy as np
import ml_dtypes

B, S, H = 16, 2048, 1024
D = 2 * H
N_CORES = 8
BPC = B // N_CORES  # 2
NT = 512
KT = D // 128       # 16 k-tiles
JT = H // 128       # 8 j-tiles

WS = 64.0           # host-side W_e scale (undone in tanh's scale arg)
NKK16 = 0           # trailing k-tiles computed in fp16 (error dial)

# Last chunks narrow: shortens the non-overlappable softmax+context tail.
CHUNKS = [[512] * 4 for _ in range(BPC)]
CHUNKS[BPC - 1] = [512, 512, 512, 384, 128]
NCH = max(len(c) for c in CHUNKS)

F8 = ml_dtypes.float8_e4m3
F16 = np.float16
BF16 = ml_dtypes.bfloat16

_cache = {}


def _build():
    import concourse.bacc as bacc
    import concourse.tile as tile
    from concourse import bass_isa, mybir

    nc = bacc.Bacc("TRN2", target_bir_lowering=False, debug=False)
    dt = mybir.dt
    DR = mybir.MatmulPerfMode.DoubleRow

    # chunk-major: for chunk ci of batch b (width w, t-offset t0), columns
    # [KT*t0 : KT*(t0+w)] hold block[p, kk*w + t] = enc[b, t0+t, kk*128+p]
    # enc8 feeds the PE (3D tile for the DoubleRow pair APs); ench feeds
    # the DVE/GpSimd context path (2D tile -- 3D APs cost ~+220ns per
    # DVE op in the reshape front-end) and the optional fp16 matmuls.
    enc8_d = nc.declare_dram_parameter(
        "enc8", [BPC, 128, KT * S], dt.float8e4, isOutput=False
    )
    ench_d = nc.declare_dram_parameter(
        "ench", [BPC, 128, KT * S], dt.float16, isOutput=False
    )
    # w8[p, kk*H + jj*128 + j] = WS * w_eT[kk*128 + p, jj*128 + j]
    w8_d = nc.declare_dram_parameter("w8", [128, KT * H], dt.float8e4, isOutput=False)
    if NKK16:
        wh_d = nc.declare_dram_parameter(
            "wh", [128, NKK16 * H], dt.float16, isOutput=False
        )
    c_cols_d = nc.declare_dram_parameter("c_cols", [128, JT], dt.float32, isOutput=False)
    # fp32: used as the per-partition scale AP of the ACT v-multiplies
    v_cols_d = nc.declare_dram_parameter("v_cols", [128, JT], dt.float32, isOutput=False)
    out_part = nc.declare_dram_parameter(
        "out_part", [BPC, 128, NCH * KT], dt.float32, isOutput=True
    )
    out_sums = nc.declare_dram_parameter(
        "out_sums", [BPC, 1, NCH], dt.float32, isOutput=True
    )
    # [t, d]-oriented fp16 enc rows for the last batch's final four
    # chunks (12 x 128 t-rows): td[p, c*D + d] = enc[BPC-1, S-1536+c*128+p, d].
    # Their context partials run on the PE instead of the DVE stt chain.
    enc_td_d = nc.declare_dram_parameter(
        "enc_td", [128, 12 * D], dt.float16, isOutput=False
    )
    out_ctxl = nc.declare_dram_parameter(
        "out_ctxl", [1, D], dt.float32, isOutput=True
    )

    AF = mybir.ActivationFunctionType
    OP = mybir.AluOpType
    NP8 = KT - NKK16          # k-tiles in fp8 (paired for DoubleRow)
    NPAIR = NP8 // 2

    with tile.TileContext(nc) as tc:
        with (
            tc.tile_pool(name="weights", bufs=1) as wpool,
            tc.tile_pool(name="enc8", bufs=3) as e8pool,
            tc.tile_pool(name="ench", bufs=3) as ehpool,
            tc.tile_pool(name="energy", bufs=3) as epool,
            tc.tile_pool(name="perb", bufs=2) as bpool,
            tc.tile_pool(name="psum_e", bufs=4, space="PSUM") as pe_pool,
            tc.tile_pool(name="psum_s", bufs=2, space="PSUM") as ps_pool,
            tc.tile_pool(name="psum_c", bufs=2, space="PSUM") as pc_pool,
        ):
            # ---- resident weights/constants -----------------------------
            w8_sb = wpool.tile([128, KT, H], dt.float8e4, tag="w8")
            if NKK16:
                wh_sb = wpool.tile([128, NKK16, H], dt.float16, tag="wh")
            c_sb = wpool.tile([128, JT], dt.float32, tag="c")
            v_sb = wpool.tile([128, JT], dt.float32, tag="v")

            def dma_w8(kk, half=None):
                lo = 0 if half != 1 else H // 2
                hi = H if half != 0 else H // 2
                nc.sync.dma_start(
                    w8_sb[:, kk, lo:hi], w8_d.ap()[:, kk * H + lo : kk * H + hi]
                )

            enc_tiles = {}
            offs = [
                [sum(CHUNKS[b][:ci]) for ci in range(len(CHUNKS[b]))]
                for b in range(BPC)
            ]

            def dma_enc(b, ci, which):
                """DMA one chunk of enc8 ('8', 3D tile) or ench ('h', 2D)."""
                w = CHUNKS[b][ci]
                c0 = KT * offs[b][ci]
                src = (enc8_d if which == "8" else ench_d).ap()[b]
                # split per kk-pair so the round-robin queue assignment
                # spreads one chunk's transfer across 8 DMA queues
                if which == "8":
                    t = e8pool.tile(
                        [128, KT, NT], dt.float8e4, tag="enc8", name=f"enc8{b}_{ci}"
                    )
                    for k0 in range(0, KT, 4):
                        nc.sync.dma_start(
                            t[:, k0 : k0 + 4, :w],
                            src[:, c0 + k0 * w : c0 + (k0 + 4) * w],
                        )
                else:
                    # 2D chunk-major [kk*w + t], exactly the DRAM layout
                    t = ehpool.tile(
                        [128, KT * NT], dt.float16, tag="ench", name=f"ench{b}_{ci}"
                    )
                    for k0 in range(0, KT, 4):
                        nc.sync.dma_start(
                            t[:, k0 * w : (k0 + 4) * w],
                            src[:, c0 + k0 * w : c0 + (k0 + 4) * w],
                        )
                enc_tiles[(b, ci, which)] = t

            # startup: interleave per-kk fp8 weight slices with the first
            # chunk's enc8 pair-blocks so the jj-low matmuls stream kk by
            # kk behind the DMA arrivals. The c/v constants go AFTER the
            # critical stream (their serialized DIRECT2D issue otherwise
            # delays it by ~2us); they are only needed at the first tanh.
            first8 = e8pool.tile([128, KT, NT], dt.float8e4, tag="enc8", name="enc80_0")
            src80 = enc8_d.ap()[0]
            w00 = CHUNKS[0][0]
            for k0 in range(0, KT, 2):
                dma_w8(k0, half=0)
                nc.sync.dma_start(
                    first8[:, k0 : k0 + 2, :w00],
                    src80[:, k0 * w00 : (k0 + 2) * w00],
                )
                dma_w8(k0 + 1, half=0)
            nc.sync.dma_start(c_sb[:], c_cols_d.ap()[:])
            nc.sync.dma_start(v_sb[:], v_cols_d.ap()[:])
            if NKK16:
                nc.sync.dma_start(wh_sb[:, :, :], wh_d.ap()[:, :])
            for kk in range(KT):
                dma_w8(kk, half=1)
            enc_tiles[(0, 0, "8")] = first8
            dma_enc(0, 1, "8")

            sums_t = {}
            part_t = {}
            for b in range(BPC):
                sums_t[b] = bpool.tile(
                    [1, NCH], dt.float32, tag="sums", name=f"sums{b}"
                )
                part_t[b] = bpool.tile(
                    [128, NCH * KT], dt.float32, tag="part", name=f"part{b}"
                )

            all_chunks = [
                (b, ci) for b in range(BPC) for ci in range(len(CHUNKS[b]))
            ]

            # tail chunks: context on the PE via a DMA-transposed u column.
            # TAIL maps chunk -> base 128-t block index (12 blocks total
            # over the last batch's final four chunks).
            TAIL = {}
            _base = 0
            for _ci in range(len(CHUNKS[BPC - 1]) - 4, len(CHUNKS[BPC - 1])):
                TAIL[(BPC - 1, _ci)] = _base
                _base += CHUNKS[BPC - 1][_ci] // 128
            NBLK = _base  # 12
            # quarter q lives at partition row 64*(q%2) of bank q//2
            # (bass caps PSUM base_partition at 64, so two banks)
            ctx_ps = [
                pc_pool.tile([128, NT], dt.float32, tag="ctxps", name=f"ctxps{i}")
                for i in range(2)
            ]
            u_col = bpool.tile([128, NBLK], dt.float16, tag="ucol", bufs=1)
            enc_td_t = wpool.tile([128, NBLK * D], dt.float16, tag="enctd")

            def finish_tail(b, ci, w, s_ps, base):
                u_row = bpool.tile([1, NT], dt.float16, tag="urow")
                nc.scalar.activation(
                    u_row[:, :w], s_ps[:, :w], AF.Exp,
                    accum_out=sums_t[b][0:1, ci : ci + 1],
                )
                # transpose u into partitions: u_col[p, base+blk] = u[blk*128+p]
                for blk in range(w // 128):
                    nc.sync.dma_start(
                        u_col[:, base + blk : base + blk + 1],
                        u_row[0:1, blk * 128 : (blk + 1) * 128],
                    )
                nch = len(CHUNKS[b])
                if ci == nch - 1:
                    nc.sync.dma_start(
                        out_sums.ap()[b][:, :nch], sums_t[b][0:1, :nch]
                    )

            def ctx_mms(base, nblk):
                # t-blocks x 4 D-quarters; quarter q accumulates at
                # partition row 64*(q%2) of bank q//2
                for blk in range(nblk):
                    c = base + blk
                    for q in range(4):
                        row = 64 * (q % 2)
                        nc.tensor.matmul(
                            ctx_ps[q // 2][row : row + 1, :NT],
                            u_col[:, c : c + 1],
                            enc_td_t[:, c * D + q * NT : c * D + (q + 1) * NT],
                            start=(c == 0),
                            stop=(c == NBLK - 1),
                            skip_group_check=True,
                        )

            pend_stt = []  # spread stt issuance across jj slots (DVE FIFO)

            def finish_chunk(b, ci, w, s_ps, ench_t):
                u_row = bpool.tile([1, NT], dt.float16, tag="urow")
                nc.scalar.activation(
                    u_row[:, :w], s_ps[:, :w], AF.Exp,
                    accum_out=sums_t[b][0:1, ci : ci + 1],
                )
                u_bc = bpool.tile([128, NT], dt.float16, tag="ubc")
                nc.gpsimd.partition_broadcast(u_bc[:, :w], u_row[:, :w])

                def stt(kk):
                    scratch = bpool.tile([128, NT], dt.bfloat16, tag="scr")
                    nc.vector.scalar_tensor_tensor(
                        out=scratch[:, :w],
                        in0=ench_t[:, kk * w : (kk + 1) * w],
                        scalar=1.0,
                        in1=u_bc[:, :w],
                        op0=OP.mult,
                        op1=OP.mult,
                        accum_out=part_t[b][:, ci * KT + kk : ci * KT + kk + 1],
                    )

                def fin():
                    nc.sync.dma_start(
                        out_part.ap()[b][:, ci * KT : (ci + 1) * KT],
                        part_t[b][:, ci * KT : (ci + 1) * KT],
                    )
                    nch = len(CHUNKS[b])
                    if ci == nch - 1:
                        nc.sync.dma_start(
                            out_sums.ap()[b][:, :nch], sums_t[b][0:1, :nch]
                        )

                pend_stt.extend(
                    [lambda kk=kk: stt(kk) for kk in range(KT)]
                )
                pend_stt.append(fin)

            def drain_stt(n):
                for _ in range(min(n, len(pend_stt))):
                    pend_stt.pop(0)()

            # Scores: the v-multiply runs on the Scalar engine (Copy with
            # per-partition fp32 scale AP) right after each tanh; the
            # jj-tree adds on DVE; the partition reduction is a single
            # ones-stationary matmul per chunk, deferred into the next
            # chunk's jj=0 slot so the PE never waits on the DVE tree.
            ones_col = wpool.tile([128, 1], dt.float16, tag="ones")
            nc.vector.memset(ones_col[:], 1.0)
            carry = None  # (b, ci, w, s_ps, esum, ench_t)
            pend_ctx = []  # deferred tail ctx-MM closures

            def fire_carry():
                nonlocal carry
                pb, pci, pw, ps_ps, pesum, pench_t = carry
                nc.tensor.matmul(
                    ps_ps[:, :pw], ones_col[:], pesum[:, :pw],
                    start=True, stop=True,
                )
                if (pb, pci) in TAIL:
                    base = TAIL[(pb, pci)]
                    finish_tail(pb, pci, pw, ps_ps, base)
                    pend_ctx.append((base, pw // 128))
                else:
                    finish_chunk(pb, pci, pw, ps_ps, pench_t)
                carry = None

            for b, ci in all_chunks:
                w = CHUNKS[b][ci]
                if (b, ci, "8") not in enc_tiles:
                    dma_enc(b, ci, "8")
                enc8_t = enc_tiles.pop((b, ci, "8"))
                # ench(c) is DMA'd lazily at this chunk's jj=2 -- it is
                # first read by finish_chunk(c) during chunk c+1, so it
                # stays off the startup critical DMA path. (NKK16 needs
                # it during the chunk itself: fetch at chunk start then.)
                if NKK16 and (b, ci, "h") not in enc_tiles:
                    dma_enc(b, ci, "h")
                ench_t = (
                    enc_tiles.pop((b, ci, "h"))
                    if (b, ci, "h") in enc_tiles
                    else None
                )
                nch = len(CHUNKS[b])
                nxt = [(b, c2) for c2 in range(ci + 1, nch)] + [
                    (b2, c2)
                    for b2 in range(b + 1, BPC)
                    for c2 in range(len(CHUNKS[b2]))
                ]
                if nxt and (nxt[0][0], nxt[0][1], "8") not in enc_tiles:
                    dma_enc(nxt[0][0], nxt[0][1], "8")
                if (b, ci) == (BPC - 1, len(CHUNKS[BPC - 1]) - 5):
                    for c4 in range(NBLK):
                        nc.sync.dma_start(
                            enc_td_t[:, c4 * D : (c4 + 1) * D],
                            enc_td_d.ap()[:, c4 * D : (c4 + 1) * D],
                        )

                s_ps = ps_pool.tile(
                    [1, NT], dt.float32, tag="sps", name=f"sps{b}_{ci}"
                )
                acc = None
                for jj in range(JT):
                    e_ps = pe_pool.tile([128, NT], dt.float32, tag="eps")
                    for pk in range(NPAIR):
                        kk0 = 2 * pk
                        nc.tensor.matmul(
                            e_ps[:, :w],
                            w8_sb[:, kk0 : kk0 + 2, jj * 128 : (jj + 1) * 128],
                            enc8_t[:, kk0 : kk0 + 2, :w],
                            start=(pk == 0),
                            stop=(pk == NPAIR - 1 and NKK16 == 0),
                            perf_mode=DR,
                        )
                    for i in range(NKK16):
                        kk = NP8 + i
                        nc.tensor.matmul(
                            e_ps[:, :w],
                            wh_sb[:, i, jj * 128 : (jj + 1) * 128],
                            ench_t[:, kk * w : (kk + 1) * w],
                            start=False,
                            stop=(i == NKK16 - 1),
                        )
                    if jj == 4 and ench_t is None and (b, ci) not in TAIL:
                        dma_enc(b, ci, "h")
                        ench_t = enc_tiles.pop((b, ci, "h"))
                    if jj == 6 and len(nxt) > 1 and (
                        nxt[1][0], nxt[1][1], "8"
                    ) not in enc_tiles:
                        dma_enc(nxt[1][0], nxt[1][1], "8")
                    if jj == 1 and carry is not None:
                        fire_carry()
                    if jj == 3 and pend_ctx:
                        ctx_mms(*pend_ctx.pop(0))
                    drain_stt(3)
                    et = epool.tile([128, NT], dt.float16, tag="et", bufs=6)
                    nc.scalar.activation(
                        et[:, :w], e_ps[:, :w], AF.Tanh,
                        bias=c_sb[:, jj : jj + 1], scale=1.0 / WS,
                    )
                    ev = bpool.tile(
                        [128, NT], dt.float16, tag="ev", bufs=3, name=f"ev{jj}"
                    )
                    nc.vector.tensor_scalar_mul(
                        ev[:, :w], et[:, :w], v_sb[:, jj : jj + 1],
                    )
                    if acc is None:
                        acc = ev
                    else:
                        nacc = bpool.tile(
                            [128, NT], dt.float16, tag="esum", bufs=3,
                            name=f"esum{jj}",
                        )
                        nc.vector.tensor_add(nacc[:, :w], acc[:, :w], ev[:, :w])
                        acc = nacc
                carry = (b, ci, w, s_ps, acc, ench_t)

            fire_carry()
            while pend_stt:
                pend_stt.pop(0)()
            while pend_ctx:
                ctx_mms(*pend_ctx.pop(0))
            # drain the tail context psum: 4 quarter rows -> one fp32 row
            ctx_row = bpool.tile([1, D], dt.float32, tag="ctxrow")
            for q in range(4):
                row = 64 * (q % 2)
                nc.scalar.activation(
                    ctx_row[0:1, q * NT : (q + 1) * NT],
                    ctx_ps[q // 2][row : row + 1, :NT], AF.Copy,
                )
            nc.sync.dma_start(out_ctxl.ap()[:, :], ctx_row[0:1, :])

    nc.compile()
    return nc


def _get_nc():
    if "nc" not in _cache:
        import time

        t0 = time.time()
        _cache["nc"] = _build()
        if os.environ.get("KERNEL_TRACE"):
            print(f"[kernel] bass build+compile: {time.time() - t0:.1f} s")
    return _cache["nc"]


def kernel(hidden, encoder_outputs, attn_w, attn_b, v_w):
    from concourse.bass_utils import run_bass_kernel_spmd

    nc = _get_nc()

    hidden = np.asarray(hidden, dtype=np.float32)
    enc = np.asarray(encoder_outputs, dtype=np.float32)
    attn_w = np.asarray(attn_w, dtype=np.float32)
    attn_b = np.asarray(attn_b, dtype=np.float32)
    v_w = np.asarray(v_w, dtype=np.float32)

    w_eT = np.ascontiguousarray(attn_w[:, D:].T) * WS            # [D, H]
    w_kk = w_eT.reshape(KT, 128, H).transpose(1, 0, 2)           # [128, KT, H]
    w8 = np.ascontiguousarray(w_kk).reshape(128, KT * H).astype(F8)
    if NKK16:
        wh = np.ascontiguousarray(w_kk[:, KT - NKK16 :]).reshape(
            128, NKK16 * H
        ).astype(F16)
    c = (hidden @ attn_w[:, :D].T + attn_b).astype(np.float32)   # [1, H]
    c_cols = np.ascontiguousarray(c.reshape(JT, 128).T)          # [128, JT]
    v_cols = np.ascontiguousarray(v_w.reshape(JT, 128).T)       # fp32

    in_maps = []
    for cidx in range(N_CORES):
        sl = enc[cidx * BPC : (cidx + 1) * BPC]                  # [BPC, S, D]
        rows = []
        for b in range(BPC):
            t0 = 0
            blocks = []
            for wdt in CHUNKS[b]:
                blk = (
                    sl[b, t0 : t0 + wdt]
                    .reshape(wdt, KT, 128)
                    .transpose(2, 1, 0)
                    .reshape(128, KT * wdt)
                )
                blocks.append(blk)
                t0 += wdt
            rows.append(np.concatenate(blocks, axis=1))
        encT2 = np.ascontiguousarray(np.stack(rows))
        # [t, d] layout for the last batch's three tail chunks
        enc_td = np.ascontiguousarray(
            sl[BPC - 1, S - 1536 :]
            .reshape(12, 128, D)
            .transpose(1, 0, 2)
            .reshape(128, 12 * D)
        ).astype(F16)
        m = {
            "enc8": encT2.astype(F8),
            "ench": encT2.astype(F16),
            "enc_td": enc_td,
            "w8": w8,
            "c_cols": c_cols,
            "v_cols": v_cols,
        }
        if NKK16:
            m["wh"] = wh
        in_maps.append(m)

    trace = bool(os.environ.get("KERNEL_TRACE"))
    if trace:
        _install_prof_shim()
    res = run_bass_kernel_spmd(
        nc, in_maps, core_ids=list(range(N_CORES)), trace=trace
    )
    if trace:
        _cache["last_exec_time_ns"] = res.exec_time_ns
        print(f"HW exec time: {res.exec_time_ns} ns")

    ctx = np.empty((B, 1, D), dtype=np.float32)
    for cidx in range(N_CORES):
        part = np.asarray(res.results[cidx]["out_part"], dtype=np.float32)
        sums = np.asarray(res.results[cidx]["out_sums"], dtype=np.float32)
        ctxl = np.asarray(res.results[cidx]["out_ctxl"], dtype=np.float32)
        for b in range(BPC):
            nch = len(CHUNKS[b])
            ne = nch - 4 if b == BPC - 1 else nch
            acc = part[b][:, : ne * KT].reshape(128, ne, KT).sum(axis=1)
            flat = acc.T.reshape(D)
            if b == BPC - 1:
                flat = flat + ctxl[0]
            ctx[cidx * BPC + b, 0, :] = flat / sums[b][0, :nch].sum()
    return ctx


def _install_prof_shim():
    """antenv.axon_hooks is absent from this image; inject it so
    run_bass_kernel_spmd(trace=True) can capture NTFF profiles."""
    import sys
    import types

    if "antenv.axon_hooks" in sys.modules:
        return
    import antenv

    mod = types.ModuleType("antenv.axon_hooks")
    mod._hook = None
    mod.set_axon_ntff_profile_hook = lambda h: setattr(mod, "_hook", h)
    mod.get_axon_ntff_profile_hook = lambda: mod._hook
    sys.modules["antenv.axon_hooks"] = mod
    antenv.axon_hooks = mod
    try:
        from trn_agent_boot.trn_boot import _ntff_profile_via_ctypes

        mod.set_axon_ntff_profile_hook(
            _ntff_profile_via_ctypes("/opt/axon/libaxon_pjrt.so")
        )
    except Exception:
        pass


# revision 83
# speedup vs baseline: 1.0663x; 1.0663x over previous
"""Bass/Trainium2 kernel for nn_Attention_21354577395789 (fp8 DoubleRow).

Reference computation (B=16, S=2048, H=1024, D=2H=2048):
    h      = broadcast(hidden[1, 2H]) -> [B, S, 2H]
    cat    = concat([h, enc], -1)                    [B, S, 4H]
    energy = tanh(cat @ attn_w.T + attn_b)           [B, S, H]
    scores = energy @ v_w.T                          [B, S, 1]
    attn   = softmax(scores, axis=1)
    ctx    = attn^T @ enc                            [B, 1, 2H]

Algebraic simplifications (as the bf16 baseline):
  * attn_w = [W_h | W_e]; c = hidden @ W_h.T + attn_b is a single [H]
    vector computed on the host.
  * streaming softmax without max subtraction; per-chunk unnormalized
    context partials + exp-sums; final reduction on host.

Precision strategy (the speedup over the bf16 baseline):
  * The O(S*D*H) energy matmul runs in fp8-e4m3 with
    perf_mode=DoubleRow: two k-slices (256 contraction rows) per PE
    pass -> ~1.8x the bf16 matmul rate. W_e is pre-scaled by WS=64 on
    the host (its std 1/64 would land in e4m3's subnormal range);
    the tanh activation applies scale=1/WS to undo it.
  * Optionally the last NKK16 k-tiles run as plain fp16 matmuls into
    the same PSUM accumulation, dialing the quantization error down
    (sim: NKK16=0 -> 1.78e-2, 2 -> 1.68e-2, 4 -> 1.55e-2 vs the 2e-2
    gate) at ~267ns per extra matmul.
  * The context path (DVE) reads a separate fp16 copy of enc, so fp8
    noise never touches the context accumulation.

Engine placement per 512-chunk of S:
  PE    : DoubleRow energy matmuls (the only O(S*D*H) work) + one
          ones-stationary score-reduce per chunk (carry-deferred into
          the next chunk's jj=1 slot)
  ACT   : tanh(+bias+1/WS scale), exp(+chunk sum)
  GpSimd: broadcast of the exp row across partitions
  DVE   : v-mult + jj-tree (fp16) and 16 per-kk context stt ops,
          issued 3 per jj slot so the strict-FIFO DVE queue never
          starves the ev/add chain that recycles tanh buffers
The last batch's final three chunks skip the DVE context path
entirely: their exp rows are DMA-transposed into partition columns
and the context partials run as PE matmuls against a [t, d]-layout
fp16 enc copy, accumulating in one PSUM readout. This keeps the
end-of-stream region (where per-chunk PE time shrinks) free of the
DVE backlog that otherwise convoys the in-order PE queue.

Distribution: data-parallel over B across 8 NeuronCores (2 batches
per core), no collectives.
"""

import os

import numpy as np
import ml_dtypes

B, S, H = 16, 2048, 1024
D = 2 * H
N_CORES = 8
BPC = B // N_CORES  # 2
NT = 512
KT = D // 128       # 16 k-tiles
JT = H // 128       # 8 j-tiles

WS = 64.0           # host-side W_e scale (undone in tanh's scale arg)
NKK16 = 0           # trailing k-tiles computed in fp16 (error dial)

# Last chunks narrow: shortens the non-overlappable softmax+context tail.
CHUNKS = [[512] * 4 for _ in range(BPC)]
CHUNKS[BPC - 1] = [512, 512, 512, 384, 128]
NCH = max(len(c) for c in CHUNKS)

F8 = ml_dtypes.float8_e4m3
F16 = np.float16
BF16 = ml_dtypes.bfloat16

_cache = {}


def _build():
    import concourse.bacc as bacc
    import concourse.tile as tile
    from concourse import bass_isa, mybir

    nc = bacc.Bacc("TRN2", target_bir_lowering=False, debug=False)
    dt = mybir.dt
    DR = mybir.MatmulPerfMode.DoubleRow

    # chunk-major: for chunk ci of batch b (width w, t-offset t0), columns
    # [KT*t0 : KT*(t0+w)] hold block[p, kk*w + t] = enc[b, t0+t, kk*128+p]
    # enc8 feeds the PE (3D tile for the DoubleRow pair APs); ench feeds
    # the DVE/GpSimd context path (2D tile -- 3D APs cost ~+220ns per
    # DVE op in the reshape front-end) and the optional fp16 matmuls.
    enc8_d = nc.declare_dram_parameter(
        "enc8", [BPC, 128, KT * S], dt.float8e4, isOutput=False
    )
    ench_d = nc.declare_dram_parameter(
        "ench", [BPC, 128, KT * S], dt.float16, isOutput=False
    )
    # w8[p, kk*H + jj*128 + j] = WS * w_eT[kk*128 + p, jj*128 + j]
    w8_d = nc.declare_dram_parameter("w8", [128, KT * H], dt.float8e4, isOutput=False)
    if NKK16:
        wh_d = nc.declare_dram_parameter(
            "wh", [128, NKK16 * H], dt.float16, isOutput=False
        )
    c_cols_d = nc.declare_dram_parameter("c_cols", [128, JT], dt.float32, isOutput=False)
    # fp32: used as the per-partition scale AP of the ACT v-multiplies
    v_cols_d = nc.declare_dram_parameter("v_cols", [128, JT], dt.float32, isOutput=False)
    out_part = nc.declare_dram_parameter(
        "out_part", [BPC, 128, NCH * KT], dt.float32, isOutput=True
    )
    out_sums = nc.declare_dram_parameter(
        "out_sums", [BPC, 1, NCH], dt.float32, isOutput=True
    )
    # [t, d]-oriented fp16 enc rows for the last batch's final four
    # chunks (12 x 128 t-rows): td[p, c*D + d] = enc[BPC-1, S-1536+c*128+p, d].
    # Their context partials run on the PE instead of the DVE stt chain.
    enc_td_d = nc.declare_dram_parameter(
        "enc_td", [128, 12 * D], dt.float16, isOutput=False
    )
    out_ctxl = nc.declare_dram_parameter(
        "out_ctxl", [1, D], dt.float32, isOutput=True
    )

    AF = mybir.ActivationFunctionType
    OP = mybir.AluOpType
    NP8 = KT - NKK16          # k-tiles in fp8 (paired for DoubleRow)
    NPAIR = NP8 // 2

    with tile.TileContext(nc) as tc:
        with (
            tc.tile_pool(name="weights", bufs=1) as wpool,
            tc.tile_pool(name="enc8", bufs=3) as e8pool,
            tc.tile_pool(name="ench", bufs=3) as ehpool,
            tc.tile_pool(name="energy", bufs=3) as epool,
            tc.tile_pool(name="perb", bufs=2) as bpool,
            tc.tile_pool(name="psum_e", bufs=4, space="PSUM") as pe_pool,
            tc.tile_pool(name="psum_s", bufs=2, space="PSUM") as ps_pool,
            tc.tile_pool(name="psum_c", bufs=2, space="PSUM") as pc_pool,
        ):
            # ---- resident weights/constants -----------------------------
            w8_sb = wpool.tile([128, KT, H], dt.float8e4, tag="w8")
            if NKK16:
                wh_sb = wpool.tile([128, NKK16, H], dt.float16, tag="wh")
            c_sb = wpool.tile([128, JT], dt.float32, tag="c")
            v_sb = wpool.tile([128, JT], dt.float32, tag="v")

            def dma_w8(kk, half=None):
                lo = 0 if half != 1 else H // 2
                hi = H if half != 0 else H // 2
                nc.sync.dma_start(
                    w8_sb[:, kk, lo:hi], w8_d.ap()[:, kk * H + lo : kk * H + hi]
                )

            enc_tiles = {}
            offs = [
                [sum(CHUNKS[b][:ci]) for ci in range(len(CHUNKS[b]))]
                for b in range(BPC)
            ]

            def dma_enc(b, ci, which):
                """DMA one chunk of enc8 ('8', 3D tile) or ench ('h', 2D)."""
                w = CHUNKS[b][ci]
                c0 = KT * offs[b][ci]
                src = (enc8_d if which == "8" else ench_d).ap()[b]
                # split per kk-pair so the round-robin queue assignment
                # spreads one chunk's transfer across 8 DMA queues
                if which == "8":
                    t = e8pool.tile(
                        [128, KT, NT], dt.float8e4, tag="enc8", name=f"enc8{b}_{ci}"
                    )
                    for k0 in range(0, KT, 4):
                        nc.sync.dma_start(
                            t[:, k0 : k0 + 4, :w],
                            src[:, c0 + k0 * w : c0 + (k0 + 4) * w],
                        )
                else:
                    # 2D chunk-major [kk*w + t], exactly the DRAM layout
                    t = ehpool.tile(
                        [128, KT * NT], dt.float16, tag="ench", name=f"ench{b}_{ci}"
                    )
                    for k0 in range(0, KT, 4):
                        nc.sync.dma_start(
                            t[:, k0 * w : (k0 + 4) * w],
                            src[:, c0 + k0 * w : c0 + (k0 + 4) * w],
                        )
                enc_tiles[(b, ci, which)] = t

            # startup: interleave per-kk fp8 weight slices with the first
            # chunk's enc8 pair-blocks so the jj-low matmuls stream kk by
            # kk behind the DMA arrivals. The c/v constants go AFTER the
            # critical stream (their serialized DIRECT2D issue otherwise
            # delays it by ~2us); they are only needed at the first tanh.
            first8 = e8pool.tile([128, KT, NT], dt.float8e4, tag="enc8", name="enc80_0")
            src80 = enc8_d.ap()[0]
            w00 = CHUNKS[0][0]
            for k0 in range(0, KT, 2):
                dma_w8(k0, half=0)
                nc.sync.dma_start(
                    first8[:, k0 : k0 + 2, :w00],
                    src80[:, k0 * w00 : (k0 + 2) * w00],
                )
                dma_w8(k0 + 1, half=0)
            nc.sync.dma_start(c_sb[:], c_cols_d.ap()[:])
            nc.sync.dma_start(v_sb[:], v_cols_d.ap()[:])
            if NKK16:
                nc.sync.dma_start(wh_sb[:, :, :], wh_d.ap()[:, :])
            for kk in range(KT):
                dma_w8(kk, half=1)
            enc_tiles[(0, 0, "8")] = first8
            dma_enc(0, 1, "8")

            sums_t = {}
            part_t = {}
            for b in range(BPC):
                sums_t[b] = bpool.tile(
                    [1, NCH], dt.float32, tag="sums", name=f"sums{b}"
                )
                part_t[b] = bpool.tile(
                    [128, NCH * KT], dt.float32, tag="part", name=f"part{b}"
                )

            all_chunks = [
                (b, ci) for b in range(BPC) for ci in range(len(CHUNKS[b]))
            ]

            # tail chunks: context on the PE via a DMA-transposed u column.
            # TAIL maps chunk -> base 128-t block index (12 blocks total
            # over the last batch's final four chunks).
            TAIL = {}
            _base = 0
            for _ci in range(len(CHUNKS[BPC - 1]) - 4, len(CHUNKS[BPC - 1])):
                TAIL[(BPC - 1, _ci)] = _base
                _base += CHUNKS[BPC - 1][_ci] // 128
            NBLK = _base  # 12
            # quarter q lives at partition row 64*(q%2) of bank q//2
            # (bass caps PSUM base_partition at 64, so two banks)
            ctx_ps = [
                pc_pool.tile([128, NT], dt.float32, tag="ctxps", name=f"ctxps{i}")
                for i in range(2)
            ]
            u_col = bpool.tile([128, NBLK], dt.float16, tag="ucol", bufs=1)
            enc_td_t = wpool.tile([128, NBLK * D], dt.float16, tag="enctd")

            def finish_tail(b, ci, w, s_ps, base):
                u_row = bpool.tile([1, NT], dt.float16, tag="urow")
                nc.scalar.activation(
                    u_row[:, :w], s_ps[:, :w], AF.Exp,
                    accum_out=sums_t[b][0:1, ci : ci + 1],
                )
                # transpose u into partitions: u_col[p, base+blk] = u[blk*128+p]
                for blk in range(w // 128):
                    nc.sync.dma_start(
                        u_col[:, base + blk : base + blk + 1],
                        u_row[0:1, blk * 128 : (blk + 1) * 128],
                    )
                nch = len(CHUNKS[b])
                if ci == nch - 1:
                    nc.sync.dma_start(
                        out_sums.ap()[b][:, :nch], sums_t[b][0:1, :nch]
                    )

            def ctx_mms(base, nblk):
                # t-blocks x 4 D-quarters; quarter q accumulates at
                # partition row 64*(q%2) of bank q//2
                for blk in range(nblk):
                    c = base + blk
                    for q in range(4):
                        row = 64 * (q % 2)
                        nc.tensor.matmul(
                            ctx_ps[q // 2][row : row + 1, :NT],
                            u_col[:, c : c + 1],
                            enc_td_t[:, c * D + q * NT : c * D + (q + 1) * NT],
                            start=(c == 0),
                            stop=(c == NBLK - 1),
                            skip_group_check=True,
                        )

            pend_stt = []  # spread stt issuance across jj slots (DVE FIFO)

            def finish_chunk(b, ci, w, s_ps, ench_t):
                u_row = bpool.tile([1, NT], dt.float16, tag="urow")
                nc.scalar.activation(
                    u_row[:, :w], s_ps[:, :w], AF.Exp,
                    accum_out=sums_t[b][0:1, ci : ci + 1],
                )
                u_bc = bpool.tile([128, NT], dt.float16, tag="ubc")
                nc.gpsimd.partition_broadcast(u_bc[:, :w], u_row[:, :w])

                def stt(kk):
                    scratch = bpool.tile([128, NT], dt.bfloat16, tag="scr")
                    nc.vector.scalar_tensor_tensor(
                        out=scratch[:, :w],
                        in0=ench_t[:, kk * w : (kk + 1) * w],
                        scalar=1.0,
                        in1=u_bc[:, :w],
                        op0=OP.mult,
                        op1=OP.mult,
                        accum_out=part_t[b][:, ci * KT + kk : ci * KT + kk + 1],
                    )

                def fin():
                    nc.sync.dma_start(
                        out_part.ap()[b][:, ci * KT : (ci + 1) * KT],
                        part_t[b][:, ci * KT : (ci + 1) * KT],
                    )
                    nch = len(CHUNKS[b])
                    if ci == nch - 1:
                        nc.sync.dma_start(
                            out_sums.ap()[b][:, :nch], sums_t[b][0:1, :nch]
                        )

                pend_stt.extend(
                    [lambda kk=kk: stt(kk) for kk in range(KT)]
                )
                pend_stt.append(fin)

            def drain_stt(n):
                for _ in range(min(n, len(pend_stt))):
                    pend_stt.pop(0)()

            # Scores: the v-multiply runs on the Scalar engine (Copy with
            # per-partition fp32 scale AP) right after each tanh; the
            # jj-tree adds on DVE; the partition reduction is a single
            # ones-stationary matmul per chunk, deferred into the next
            # chunk's jj=0 slot so the PE never waits on the DVE tree.
            ones_col = wpool.tile([128, 1], dt.float16, tag="ones")
            nc.vector.memset(ones_col[:], 1.0)
            carry = None  # (b, ci, w, s_ps, esum, ench_t)
            pend_ctx = []  # deferred tail ctx-MM closures

            def fire_carry():
                nonlocal carry
                pb, pci, pw, ps_ps, pesum, pench_t = carry
                nc.tensor.matmul(
                    ps_ps[:, :pw], ones_col[:], pesum[:, :pw],
                    start=True, stop=True,
                )
                if (pb, pci) in TAIL:
                    base = TAIL[(pb, pci)]
                    finish_tail(pb, pci, pw, ps_ps, base)
                    pend_ctx.append((base, pw // 128))
                else:
                    finish_chunk(pb, pci, pw, ps_ps, pench_t)
                carry = None

            for b, ci in all_chunks:
                w = CHUNKS[b][ci]
                if (b, ci, "8") not in enc_tiles:
                    dma_enc(b, ci, "8")
                enc8_t = enc_tiles.pop((b, ci, "8"))
                # ench(c) is DMA'd lazily at this chunk's jj=2 -- it is
                # first read by finish_chunk(c) during chunk c+1, so it
                # stays off the startup critical DMA path. (NKK16 needs
                # it during the chunk itself: fetch at chunk start then.)
                if NKK16 and (b, ci, "h") not in enc_tiles:
                    dma_enc(b, ci, "h")
                ench_t = (
                    enc_tiles.pop((b, ci, "h"))
                    if (b, ci, "h") in enc_tiles
                    else None
                )
                nch = len(CHUNKS[b])
                nxt = [(b, c2) for c2 in range(ci + 1, nch)] + [
                    (b2, c2)
                    for b2 in range(b + 1, BPC)
                    for c2 in range(len(CHUNKS[b2]))
                ]
                for pb, pc in nxt[:2]:
                    if (pb, pc, "8") not in enc_tiles:
                        dma_enc(pb, pc, "8")
                if (b, ci) == (BPC - 1, len(CHUNKS[BPC - 1]) - 5):
                    for c4 in range(NBLK):
                        nc.sync.dma_start(
                            enc_td_t[:, c4 * D : (c4 + 1) * D],
                            enc_td_d.ap()[:, c4 * D : (c4 + 1) * D],
                        )

                s_ps = ps_pool.tile(
                    [1, NT], dt.float32, tag="sps", name=f"sps{b}_{ci}"
                )
                acc = None
                for jj in range(JT):
                    e_ps = pe_pool.tile([128, NT], dt.float32, tag="eps")
                    for pk in range(NPAIR):
                        kk0 = 2 * pk
                        nc.tensor.matmul(
                            e_ps[:, :w],
                            w8_sb[:, kk0 : kk0 + 2, jj * 128 : (jj + 1) * 128],
                            enc8_t[:, kk0 : kk0 + 2, :w],
                            start=(pk == 0),
                            stop=(pk == NPAIR - 1 and NKK16 == 0),
                            perf_mode=DR,
                        )
                    for i in range(NKK16):
                        kk = NP8 + i
                        nc.tensor.matmul(
                            e_ps[:, :w],
                            wh_sb[:, i, jj * 128 : (jj + 1) * 128],
                            ench_t[:, kk * w : (kk + 1) * w],
                            start=False,
                            stop=(i == NKK16 - 1),
                        )
                    if jj == 2 and ench_t is None and (b, ci) not in TAIL:
                        dma_enc(b, ci, "h")
                        ench_t = enc_tiles.pop((b, ci, "h"))
                    if jj == 1 and carry is not None:
                        fire_carry()
                    if jj == 3 and pend_ctx:
                        ctx_mms(*pend_ctx.pop(0))
                    drain_stt(3)
                    et = epool.tile([128, NT], dt.float16, tag="et", bufs=6)
                    nc.scalar.activation(
                        et[:, :w], e_ps[:, :w], AF.Tanh,
                        bias=c_sb[:, jj : jj + 1], scale=1.0 / WS,
                    )
                    ev = bpool.tile(
                        [128, NT], dt.float16, tag="ev", bufs=3, name=f"ev{jj}"
                    )
                    nc.vector.tensor_scalar_mul(
                        ev[:, :w], et[:, :w], v_sb[:, jj : jj + 1],
                    )
                    if acc is None:
                        acc = ev
                    else:
                        nacc = bpool.tile(
                            [128, NT], dt.float16, tag="esum", bufs=3,
                            name=f"esum{jj}",
                        )
                        nc.vector.tensor_add(nacc[:, :w], acc[:, :w], ev[:, :w])
                        acc = nacc
                carry = (b, ci, w, s_ps, acc, ench_t)

            fire_carry()
            while pend_stt:
                pend_stt.pop(0)()
            while pend_ctx:
                ctx_mms(*pend_ctx.pop(0))
            # drain the tail context psum: 4 quarter rows -> one fp32 row
            ctx_row = bpool.tile([1, D], dt.float32, tag="ctxrow")
            for q in range(4):
                row = 64 * (q % 2)
                nc.scalar.activation(
                    ctx_row[0:1, q * NT : (q + 1) * NT],
                    ctx_ps[q // 2][row : row + 1, :NT], AF.Copy,
                )
            nc.sync.dma_start(out_ctxl.ap()[:, :], ctx_row[0:1, :])

    nc.compile()
    return nc


def _get_nc():
    if "nc" not in _cache:
        import time

        t0 = time.time()
        _cache["nc"] = _build()
        if os.environ.get("KERNEL_TRACE"):
            print(f"[kernel] bass build+compile: {time.time() - t0:.1f} s")
    return _cache["nc"]


def kernel(hidden, encoder_outputs, attn_w, attn_b, v_w):
    from concourse.bass_utils import run_bass_kernel_spmd

    nc = _get_nc()

    hidden = np.asarray(hidden, dtype=np.float32)
    enc = np.asarray(encoder_outputs, dtype=np.float32)
    attn_w = np.asarray(attn_w, dtype=np.float32)
    attn_b = np.asarray(attn_b, dtype=np.float32)
    v_w = np.asarray(v_w, dtype=np.float32)

    w_eT = np.ascontiguousarray(attn_w[:, D:].T) * WS            # [D, H]
    w_kk = w_eT.reshape(KT, 128, H).transpose(1, 0, 2)           # [128, KT, H]
    w8 = np.ascontiguousarray(w_kk).reshape(128, KT * H).astype(F8)
    if NKK16:
        wh = np.ascontiguousarray(w_kk[:, KT - NKK16 :]).reshape(
            128, NKK16 * H
        ).astype(F16)
    c = (hidden @ attn_w[:, :D].T + attn_b).astype(np.float32)   # [1, H]
    c_cols = np.ascontiguousarray(c.reshape(JT, 128).T)          # [128, JT]
    v_cols = np.ascontiguousarray(v_w.reshape(JT, 128).T)       # fp32

    in_maps = []
    for cidx in range(N_CORES):
        sl = enc[cidx * BPC : (cidx + 1) * BPC]                  # [BPC, S, D]
        rows = []
        for b in range(BPC):
            t0 = 0
            blocks = []
            for wdt in CHUNKS[b]:
                blk = (
                    sl[b, t0 : t0 + wdt]
                    .reshape(wdt, KT, 128)
                    .transpose(2, 1, 0)
                    .reshape(128, KT * wdt)
                )
                blocks.append(blk)
                t0 += wdt
            rows.append(np.concatenate(blocks, axis=1))
        encT2 = np.ascontiguousarray(np.stack(rows))
        # [t, d] layout for the last batch's three tail chunks
        enc_td = np.ascontiguousarray(
            sl[BPC - 1, S - 1536 :]
            .reshape(12, 128, D)
            .transpose(1, 0, 2)
            .reshape(128, 12 * D)
        ).astype(F16)
        m = {
            "enc8": encT2.astype(F8),
            "ench": encT2.astype(F16),
            "enc_td": enc_td,
            "w8": w8,
            "c_cols": c_cols,
            "v_cols": v_cols,
        }
        if NKK16:
            m["wh"] = wh
        in_maps.append(m)

    trace = bool(os.environ.get("KERNEL_TRACE"))
    if trace:
        _install_prof_shim()
    res = run_bass_kernel_spmd(
        nc, in_maps, core_ids=list(range(N_CORES)), trace=trace
    )
    if trace:
        _cache["last_exec_time_ns"] = res.exec_time_ns
        print(f"HW exec time: {res.exec_time_ns} ns")

    ctx = np.empty((B, 1, D), dtype=np.float32)
    for cidx in range(N_CORES):
        part = np.asarray(res.results[cidx]["out_part"], dtype=np.float32)
        sums = np.asarray(res.results[cidx]["out_sums"], dtype=np.float32)
        ctxl = np.asarray(res.results[cidx]["out_ctxl"], dtype=np.float32)
        for b in range(BPC):
            nch = len(CHUNKS[b])
            ne = nch - 4 if b == BPC - 1 else nch
            acc = part[b][:, : ne * KT].reshape(128, ne, KT).sum(axis=1)
            flat = acc.T.reshape(D)
            if b == BPC - 1:
                flat = flat + ctxl[0]
            ctx[cidx * BPC + b, 0, :] = flat / sums[b][0, :nch].sum()
    return ctx


def _install_prof_shim():
    """antenv.axon_hooks is absent from this image; inject it so
    run_bass_kernel_spmd(trace=True) can capture NTFF profiles."""
    import sys
    import types

    if "antenv.axon_hooks" in sys.modules:
        return
    import antenv

    mod = types.ModuleType("antenv.axon_hooks")
    mod._hook = None
    mod.set_axon_ntff_profile_hook = lambda h: setattr(mod, "_hook", h)
    mod.get_axon_ntff_profile_hook = lambda: mod._hook
    sys.modules["antenv.axon_hooks"] = mod
    antenv.axon_hooks = mod
    try:
        from trn_agent_boot.trn_boot import _ntff_profile_via_ctypes

        mod.set_axon_ntff_profile_hook(
            _ntff_profile_via_ctypes("/opt/axon/libaxon_pjrt.so")
        )
    except Exception:
        pass
